# revision 21
# baseline (speedup 1.0000x reference)
"""GraphWaveNet kernel for Trainium2 (Bass/Tile), 8 NeuronCores.

v2: edge sharding by SOURCE block + ReduceScatter (no AllGathers).

Algorithm notes:
- Only t=11 survives the final 1x1 conv; GCN doesn't mix time. So conv
  stack evaluated at t in {10,11} only, GCN on B=4 slices (D=256 cols =
  4 slices x 64 feats).
- GCN: with Hs = dsq*h, agg_n = dsq_n * (sum_{e->n} Hs[src_e] + Hs[n]).
- Sharding: core k owns node rows [1280k, 1280(k+1)). Conv computes the
  local shard of the Hs table (no collective). Edges are assigned to the
  core owning their SRC: gathers hit only the local table shard.
  Scatter (one-hot P matmuls into per-dst-block PSUM) produces a partial
  aggregate over ALL 10240 nodes; one f32 ReduceScatter per layer sums
  partials and hands each core its own 1280-row slice. Self-loop + W +
  bias + relu are then local.
- SPMD uniformity: slot schedule per dst block = max over cores of
  ceil(edges/128) (program identical across cores; per-core eidx/P data
  differ, padded with index-0 gathers and zero P columns).
- Gathers: 16 slots (2048 edges) per indirect DMA to amortize SWDGE
  descriptor-gen overhead (~1us/instruction on the Pool engine).
"""

import sys

sys.path.insert(0, "/opt/trn_rl_repo")

import numpy as np
import ml_dtypes

B, T, N, FIN, H, E = 4, 12, 10000, 2, 64, 80000
NCORES = 8
NB80 = 80                 # dst blocks of 128 nodes
NP = NB80 * 128           # padded node count (10240)
NSH = NP // NCORES        # node rows per core (1280)
NBC = NB80 // NCORES      # node blocks per core (10)
D = 4 * H                 # 256 = 4 slices x 64 feats
SPG = 8                   # slots (of 128 edges) per dma_gather (1024 idxs)
NIG = SPG * 128           # indices per gather instruction
ICPG = NIG // 16          # idx tile columns per gather instruction (64)

_cache = {}


def _host_prep(x, edge_index, w1, b1, w2, b2, gw1, gb1, gw2, gb2, wo, bo):
    x = np.asarray(x, np.float32)
    src = np.asarray(edge_index[0]).astype(np.int64)
    dst = np.asarray(edge_index[1]).astype(np.int64)

    deg = np.bincount(dst, minlength=N).astype(np.float64) + 1.0
    dsq = (deg ** -0.5).astype(np.float32)
    dsq_pad = np.ones(NP, dtype=np.float32)
    dsq_pad[:N] = dsq

    # ---- per-core edge partition by src owner, dst-sorted
    owner = src // NSH
    es_k, ed_k, cnt = [], [], np.zeros((NCORES, NB80), np.int64)
    for k in range(NCORES):
        m = owner == k
        es, ed = src[m], dst[m]
        o = np.argsort(ed, kind="stable")
        es, ed = es[o], ed[o]
        es_k.append(es)
        ed_k.append(ed)
        cnt[k] = np.bincount(ed // 128, minlength=NB80)

    S_b = np.maximum(1, (cnt + 127) // 128).max(axis=0)   # slots per block
    slots = []                                            # (block, first, last)
    for b in range(NB80):
        for j in range(int(S_b[b])):
            slots.append((b, j == 0, j == int(S_b[b]) - 1))
    TOT = len(slots)
    NGI = (TOT + SPG - 1) // SPG

    # eidx layout (dma_gather ucode contract, queue 0): within gather
    # instruction gi, flat index j in [0, NIG) lives at SBUF position
    # [16 + (j % 16), gi * ICPG + j // 16]; edge j lands at out[j%128, j//128].
    eidx_all = np.zeros((NCORES, 128, NGI * ICPG), np.int16)
    P_all = np.zeros((NCORES, 128, TOT * 128), np.float32)
    for k in range(NCORES):
        es, ed = es_k[k], ed_k[k]
        bounds = np.searchsorted(ed, np.arange(NB80 + 1) * 128)
        si = 0
        for b in range(NB80):
            e0, e1 = int(bounds[b]), int(bounds[b + 1])
            for j in range(int(S_b[b])):
                a = e0 + j * 128
                bb = min(a + 128, e1)
                ne = max(0, bb - a)
                if ne:
                    gi, c = divmod(si, SPG)
                    jj = c * 128 + np.arange(ne)        # flat idx within instr
                    eidx_all[k, 16 + (jj % 16), gi * ICPG + jj // 16] = \
                        (es[a:bb] - k * NSH).astype(np.int16)
                    P_all[k, np.arange(ne), si * 128 + (ed[a:bb] - b * 128)] = 1.0
                si += 1
    P_all = P_all.astype(ml_dtypes.bfloat16)

    # ---- conv input: per block 8 rows (t,c) for t in {9,10,11} + 2 zero rows,
    # cols = 4 slices x 128 nodes
    xpad = np.zeros((B, 3, FIN, NP), np.float32)
    xpad[:, :, :, :N] = x[:, 9:12, :, :].transpose(0, 1, 3, 2)  # [s, ti, c, n]
    xv = xpad.reshape(B, 6, NCORES, NBC, 128)                   # [s, row, k, blk, p]
    xt_all = np.zeros((NCORES, 8, NBC * 4 * 128), np.float32)
    xt_all[:, :6] = xv.transpose(2, 1, 3, 0, 4).reshape(NCORES, 6, NBC * 4 * 128)

    dsqk_all = dsq_pad.reshape(NCORES, NBC, 128).transpose(0, 2, 1).copy()

    # ---- weights
    W1m = np.zeros((6, 64), np.float32)
    for kk in range(3):
        for c in range(FIN):
            W1m[2 * kk + c, :] = w1[:, c, 0, kk]
    W1ab = np.zeros((8, 128), np.float32)
    W1ab[0:6, 0:64] = W1m          # A: t10 (taps t9,t10,t11)
    W1ab[2:8, 64:128] = W1m        # B: t11 (taps t10,t11,t12=pad)

    W2m = np.zeros((128, 64), np.float32)
    W2m[:64, :] = w2[:, :, 0, 0].T
    W2m[64:, :] = w2[:, :, 0, 1].T

    b1s = np.concatenate([b1, b1]).reshape(128, 1).astype(np.float32)
    b2c = np.asarray(b2, np.float32).reshape(64, 1)
    gb1s = np.concatenate([gb1, gb1]).reshape(128, 1).astype(np.float32)
    gb2s = np.concatenate([gb2, gb2]).reshape(128, 1).astype(np.float32)
    gwd1 = np.concatenate([gw1, gw1], axis=0).astype(ml_dtypes.bfloat16)
    gwd2 = np.concatenate([gw2, gw2], axis=0).astype(ml_dtypes.bfloat16)
    wov = np.asarray(wo, np.float32)[0, :, 0, 0]
    wod2 = np.zeros((128, 2), np.float32)
    wod2[0:64, 0] = wov
    wod2[64:128, 1] = wov
    wod2 = wod2.astype(ml_dtypes.bfloat16)

    shared = {
        "W1ab": W1ab, "W2m": W2m, "b1s": b1s, "b2c": b2c,
        "gb1s": gb1s, "gb2s": gb2s, "gwd1": gwd1, "gwd2": gwd2, "wod2": wod2,
    }
    in_maps = []
    for k in range(NCORES):
        m = dict(shared)
        m["xt"] = xt_all[k]
        m["eidx"] = eidx_all[k]
        m["P"] = P_all[k]
        m["dsqk"] = dsqk_all[k]
        in_maps.append(m)
    return in_maps, slots, NGI, float(np.asarray(bo).reshape(-1)[0])


def _build(slots, NGI, bo_f):
    from concourse import bass, bacc, tile
    from concourse.masks import make_identity
    import mybir

    f32, bf16, i16 = mybir.dt.float32, mybir.dt.bfloat16, mybir.dt.int16
    f32r = mybir.dt.float32r
    TOT = len(slots)
    COLS = NGI * ICPG

    nc = bacc.Bacc("TRN2", target_bir_lowering=False, debug=False, num_devices=8,
                   dynamic_dma_scratch_size=65536)

    ext = {}
    for name, shape, dt in [
        ("xt", [8, NBC * 512], f32), ("W1ab", [8, 128], f32),
        ("W2m", [128, 64], f32), ("b1s", [128, 1], f32), ("b2c", [64, 1], f32),
        ("gb1s", [128, 1], f32), ("gb2s", [128, 1], f32),
        ("gwd1", [128, 64], bf16), ("gwd2", [128, 64], bf16),
        ("wod2", [128, 2], bf16), ("dsqk", [128, NBC], f32),
        ("eidx", [128, COLS], i16), ("P", [128, TOT * 128], bf16),
    ]:
        ext[name] = nc.dram_tensor(name, shape, dt, kind="ExternalInput").ap()
    y_ext = nc.dram_tensor("y", [128, 4 * NBC], f32, kind="ExternalOutput").ap()
    table0 = nc.dram_tensor("table0", [NSH, D], bf16).ap()
    table1 = nc.dram_tensor("table1", [NSH, D], bf16).ap()
    aggN = [nc.dram_tensor(f"aggN{L}", [NP, D], f32).ap() for L in range(2)]
    aggS = [nc.dram_tensor(f"aggS{L}", [NSH, D], f32).ap() for L in range(2)]

    with tile.TileContext(nc) as tc:
        with tc.tile_pool(name="const", bufs=1) as cp, \
             tc.tile_pool(name="hs", bufs=1) as hp:
            ct = {}
            for name in ("W1ab", "W2m", "b1s", "b2c", "gb1s", "gb2s",
                         "gwd1", "gwd2", "wod2", "dsqk", "eidx", "P"):
                t = cp.tile(list(ext[name].shape), ext[name].dtype, tag=name)
                nc.sync.dma_start(t[:], ext[name][:])
                ct[name] = t
            ident = cp.tile([128, 128], bf16, tag="ident")
            make_identity(nc, ident[:])
            y_nb = cp.tile([128, 4 * NBC], f32, tag="ynb")

            hs0 = hp.tile([128, NBC * D], bf16, tag="hs0")
            hs1 = hp.tile([128, NBC * D], bf16, tag="hs1")
            agg_sb0 = hp.tile([128, NBC * D], f32, tag="asb0")
            agg_sb1 = hp.tile([128, NBC * D], f32, tag="asb1")
            agg_sb = [agg_sb0, agg_sb1]

            # ---- conv stage: local table0 shard = dsq * relu(conv2(relu(conv1 x)))
            with tc.tile_pool(name="cv", bufs=3) as vp, \
                 tc.tile_pool(name="cvp", bufs=2, space="PSUM") as pp:
                xts = vp.tile([8, NBC * 512], f32, tag="xts")
                nc.sync.dma_start(xts[:], ext["xt"][:])
                for blk in range(NBC):
                    ph1 = pp.tile([128, 512], f32, tag="ph1", space="PSUM")
                    nc.tensor.matmul(ph1[:], lhsT=ct["W1ab"][:],
                                     rhs=xts[:, blk * 512:(blk + 1) * 512],
                                     start=True, stop=True)
                    h1 = vp.tile([128, 512], f32, tag="h1")
                    nc.scalar.activation(h1[:], ph1[:],
                                         mybir.ActivationFunctionType.Relu,
                                         bias=ct["b1s"][:, 0:1])
                    ph2 = pp.tile([64, 512], f32, tag="ph2", space="PSUM")
                    nc.tensor.matmul(ph2[:], lhsT=ct["W2m"][:], rhs=h1[:],
                                     start=True, stop=True)
                    h2 = vp.tile([64, 512], bf16, tag="h2")
                    nc.scalar.activation(h2[:], ph2[:],
                                         mybir.ActivationFunctionType.Relu,
                                         bias=ct["b2c"][:, 0:1])
                    for s in range(4):
                        ptp = pp.tile([128, 64], bf16, tag="ptp", space="PSUM")
                        nc.tensor.transpose(ptp[:], h2[:, s * 128:(s + 1) * 128],
                                            ident[0:64, 0:64])
                        nc.vector.tensor_scalar_mul(
                            hs0[:, blk * D + 64 * s: blk * D + 64 * (s + 1)],
                            ptp[:], ct["dsqk"][:, blk:blk + 1])
                nc.sync.dma_start(
                    table0.rearrange("(lb p) f -> p lb f", p=128), hs0[:])

            # ---- GCN layers
            for L in range(2):
                tbl = table0 if L == 0 else table1
                hs_cur = hs0 if L == 0 else hs1
                gwd = ct["gwd1"] if L == 0 else ct["gwd2"]
                gbs = ct["gb1s"] if L == 0 else ct["gb2s"]

                # scatter: partial aggregate over all NP dst rows
                # blocks grouped by 4: one PSUM tile [128, 4, 512] f32 --
                # each block's accumulator bank-aligned (matmul PSUM outputs
                # must start at a bank boundary); one copy + one DMA per group
                GB = 4
                with tc.tile_pool(name=f"g{L}", bufs=5) as gp, \
                     tc.tile_pool(name=f"st{L}", bufs=4) as sp, \
                     tc.tile_pool(name=f"sc{L}", bufs=3, space="PSUM") as qp:
                    g = None
                    pb4 = None
                    for i, (b, first, last) in enumerate(slots):
                        gi, j = divmod(i, SPG)
                        if j == 0:
                            g = gp.tile([128, SPG, D], bf16, tag="g")
                            nc.gpsimd.dma_gather(
                                g[:], tbl[:],
                                ct["eidx"][:, gi * ICPG:(gi + 1) * ICPG],
                                NIG, NIG, D)
                        bg, bo_ = divmod(b, GB)
                        if first and bo_ == 0:
                            pb4 = qp.tile([128, GB, D], f32, tag="pb4",
                                          space="PSUM")
                        nc.tensor.matmul(pb4[:, bo_, 0:D],
                                         lhsT=ct["P"][:, i * 128:(i + 1) * 128],
                                         rhs=g[:, j, :],
                                         start=first, stop=last)
                        if last and bo_ == GB - 1:
                            stg = sp.tile([128, GB * D], f32, tag="stg")
                            if bg % 2 == 0:
                                nc.vector.tensor_copy(stg[:], pb4[:, :, 0:D])
                            else:
                                nc.scalar.activation(
                                    stg[:], pb4[:, :, 0:D],
                                    mybir.ActivationFunctionType.Copy)
                            nc.sync.dma_start(
                                aggN[L][bg * GB * 128:(bg + 1) * GB * 128, :]
                                .rearrange("(q p) f -> p q f", p=128),
                                stg[:])

                nc.gpsimd.collective_compute(
                    "ReduceScatter", mybir.AluOpType.add,
                    replica_groups=[list(range(NCORES))],
                    ins=[aggN[L][:]], outs=[aggS[L][:]])

                # finish: self-loop + dsq + W + bias + relu (local shard)
                with tc.tile_pool(name=f"fv{L}", bufs=3) as fv, \
                     tc.tile_pool(name=f"fp{L}", bufs=2, space="PSUM") as fp:
                    nc.sync.dma_start(
                        agg_sb[L][:],
                        aggS[L].rearrange("(lb p) f -> p lb f", p=128))
                    for lb in range(NBC):
                        ta = fv.tile([128, D], f32, tag="ta")
                        nc.vector.tensor_add(ta[:],
                                             agg_sb[L][:, lb * D:(lb + 1) * D],
                                             hs_cur[:, lb * D:(lb + 1) * D])
                        tsc = fv.tile([128, D], bf16, tag="tsc")
                        nc.vector.tensor_scalar_mul(tsc[:], ta[:],
                                                    ct["dsqk"][:, lb:lb + 1])
                        for pr in range(2):
                            tp = fp.tile([128, 128], bf16, tag="tp", space="PSUM")
                            nc.tensor.transpose(
                                tp[:], tsc[:, 128 * pr:128 * (pr + 1)], ident[:])
                            tps = fv.tile([128, 128], bf16, tag="tps")
                            if pr == 0:
                                nc.scalar.activation(
                                    tps[:], tp[:],
                                    mybir.ActivationFunctionType.Copy)
                            else:
                                nc.vector.tensor_copy(tps[:], tp[:])
                            wp = fp.tile([128, 128], f32, tag="wp", space="PSUM")
                            nc.tensor.matmul(wp[0:64, :], lhsT=gwd[0:64, :],
                                             rhs=tps[0:64, :], start=True, stop=True)
                            nc.tensor.matmul(wp[64:128, :], lhsT=gwd[64:128, :],
                                             rhs=tps[64:128, :], start=True, stop=True)
                            if L == 0:
                                hn = fv.tile([128, 128], bf16, tag="hn")
                                nc.scalar.activation(
                                    hn[:], wp[:],
                                    mybir.ActivationFunctionType.Relu,
                                    bias=gbs[:, 0:1])
                                tb = fp.tile([128, 128], bf16, tag="tb",
                                             space="PSUM")
                                nc.tensor.transpose(tb[:], hn[:], ident[:])
                                nc.vector.tensor_scalar_mul(
                                    hs1[:, lb * D + 128 * pr:
                                        lb * D + 128 * (pr + 1)],
                                    tb[:], ct["dsqk"][:, lb:lb + 1])
                            else:
                                h4 = fv.tile([128, 128], bf16, tag="h4")
                                nc.scalar.activation(
                                    h4[:], wp[:],
                                    mybir.ActivationFunctionType.Relu,
                                    bias=gbs[:, 0:1])
                                yp = fp.tile([128, 2], f32, tag="yp",
                                             space="PSUM")
                                nc.tensor.matmul(yp[:], lhsT=h4[:],
                                                 rhs=ct["wod2"][:],
                                                 start=True, stop=True)
                                nc.vector.tensor_scalar_add(
                                    y_nb[:, lb * 4 + pr * 2: lb * 4 + pr * 2 + 2],
                                    yp[:], bo_f)
                    if L == 0:
                        nc.sync.dma_start(
                            table1.rearrange("(lb p) f -> p lb f", p=128),
                            hs1[:])
            nc.sync.dma_start(y_ext[:], y_nb[:])
    nc.compile()
    return nc


def _run(inputs):
    from concourse.bass_utils import run_bass_kernel_spmd

    in_maps, slots, NGI, bo_f = _host_prep(
        inputs["x"], inputs["edge_index"], inputs["w1"], inputs["b1"],
        inputs["w2"], inputs["b2"], inputs["gw1"], inputs["gb1"],
        inputs["gw2"], inputs["gb2"], inputs["wo"], inputs["bo"])

    key = (hash(tuple(slots)), NGI)
    if key not in _cache:
        _cache[key] = _build(slots, NGI, bo_f)
    nc = _cache[key]

    res = run_bass_kernel_spmd(nc, in_maps, list(range(8)))
    y = np.zeros((B, N), dtype=np.float32)
    for k in range(NCORES):
        y_nb = res.results[k]["y"]          # [128, 4*NBC]
        for lb in range(NBC):
            lo = k * NSH + lb * 128
            hi = min(lo + 128, N)
            if hi <= lo:
                continue
            for s in range(B):
                y[s, lo:hi] = y_nb[: hi - lo, lb * 4 + s]
    return y


def kernel(**inputs):
    return _run(inputs)


# revision 22
# speedup vs baseline: 1.2392x; 1.2392x over previous
"""GraphWaveNet kernel for Trainium2 (Bass/Tile), 8 NeuronCores.

v2: edge sharding by SOURCE block + ReduceScatter (no AllGathers).

Algorithm notes:
- Only t=11 survives the final 1x1 conv; GCN doesn't mix time. So conv
  stack evaluated at t in {10,11} only, GCN on B=4 slices (D=256 cols =
  4 slices x 64 feats).
- GCN: with Hs = dsq*h, agg_n = dsq_n * (sum_{e->n} Hs[src_e] + Hs[n]).
- Sharding: core k owns node rows [1280k, 1280(k+1)). Conv computes the
  local shard of the Hs table (no collective). Edges are assigned to the
  core owning their SRC: gathers hit only the local table shard.
  Scatter (one-hot P matmuls into per-dst-block PSUM) produces a partial
  aggregate over ALL 10240 nodes; one f32 ReduceScatter per layer sums
  partials and hands each core its own 1280-row slice. Self-loop + W +
  bias + relu are then local.
- SPMD uniformity: slot schedule per dst block = max over cores of
  ceil(edges/128) (program identical across cores; per-core eidx/P data
  differ, padded with index-0 gathers and zero P columns).
- Gathers: 16 slots (2048 edges) per indirect DMA to amortize SWDGE
  descriptor-gen overhead (~1us/instruction on the Pool engine).
"""

import sys

sys.path.insert(0, "/opt/trn_rl_repo")

import numpy as np
import ml_dtypes

B, T, N, FIN, H, E = 4, 12, 10000, 2, 64, 80000
NCORES = 8
NB80 = 80                 # dst blocks of 128 nodes
NP = NB80 * 128           # padded node count (10240)
NSH = NP // NCORES        # node rows per core (1280)
NBC = NB80 // NCORES      # node blocks per core (10)
D = 4 * H                 # 256 = 4 slices x 64 feats
SPG = 8                   # slots (of 128 edges) per dma_gather (1024 idxs)
NIG = SPG * 128           # indices per gather instruction
ICPG = NIG // 16          # idx tile columns per gather instruction (64)

_cache = {}


def _host_prep(x, edge_index, w1, b1, w2, b2, gw1, gb1, gw2, gb2, wo, bo):
    x = np.asarray(x, np.float32)
    src = np.asarray(edge_index[0]).astype(np.int64)
    dst = np.asarray(edge_index[1]).astype(np.int64)

    deg = np.bincount(dst, minlength=N).astype(np.float64) + 1.0
    dsq = (deg ** -0.5).astype(np.float32)
    dsq_pad = np.ones(NP, dtype=np.float32)
    dsq_pad[:N] = dsq

    # ---- per-core edge partition by src owner, dst-sorted
    owner = src // NSH
    es_k, ed_k, cnt = [], [], np.zeros((NCORES, NB80), np.int64)
    for k in range(NCORES):
        m = owner == k
        es, ed = src[m], dst[m]
        o = np.argsort(ed, kind="stable")
        es, ed = es[o], ed[o]
        es_k.append(es)
        ed_k.append(ed)
        cnt[k] = np.bincount(ed // 128, minlength=NB80)

    S_b = np.maximum(1, (cnt + 127) // 128).max(axis=0)   # slots per block
    slots = []                                            # (block, first, last)
    for b in range(NB80):
        for j in range(int(S_b[b])):
            slots.append((b, j == 0, j == int(S_b[b]) - 1))
    TOT = len(slots)
    NGI = (TOT + SPG - 1) // SPG

    # eidx layout (dma_gather ucode contract, queue 0): within gather
    # instruction gi, flat index j in [0, NIG) lives at SBUF position
    # [16 + (j % 16), gi * ICPG + j // 16]; edge j lands at out[j%128, j//128].
    eidx_all = np.zeros((NCORES, 128, NGI * ICPG), np.int16)
    P_all = np.zeros((NCORES, 128, TOT * 128), np.float32)
    for k in range(NCORES):
        es, ed = es_k[k], ed_k[k]
        bounds = np.searchsorted(ed, np.arange(NB80 + 1) * 128)
        si = 0
        for b in range(NB80):
            e0, e1 = int(bounds[b]), int(bounds[b + 1])
            for j in range(int(S_b[b])):
                a = e0 + j * 128
                bb = min(a + 128, e1)
                ne = max(0, bb - a)
                if ne:
                    gi, c = divmod(si, SPG)
                    jj = c * 128 + np.arange(ne)        # flat idx within instr
                    eidx_all[k, 16 + (jj % 16), gi * ICPG + jj // 16] = \
                        (es[a:bb] - k * NSH).astype(np.int16)
                    P_all[k, np.arange(ne), si * 128 + (ed[a:bb] - b * 128)] = 1.0
                si += 1
    P_all = P_all.astype(ml_dtypes.bfloat16)

    # ---- conv input: per block 8 rows (t,c) for t in {9,10,11} + 2 zero rows,
    # cols = 4 slices x 128 nodes
    xpad = np.zeros((B, 3, FIN, NP), np.float32)
    xpad[:, :, :, :N] = x[:, 9:12, :, :].transpose(0, 1, 3, 2)  # [s, ti, c, n]
    xv = xpad.reshape(B, 6, NCORES, NBC, 128)                   # [s, row, k, blk, p]
    xt_all = np.zeros((NCORES, 8, NBC * 4 * 128), np.float32)
    xt_all[:, :6] = xv.transpose(2, 1, 3, 0, 4).reshape(NCORES, 6, NBC * 4 * 128)

    dsqk_all = dsq_pad.reshape(NCORES, NBC, 128).transpose(0, 2, 1).copy()

    # ---- weights
    W1m = np.zeros((6, 64), np.float32)
    for kk in range(3):
        for c in range(FIN):
            W1m[2 * kk + c, :] = w1[:, c, 0, kk]
    W1ab = np.zeros((8, 128), np.float32)
    W1ab[0:6, 0:64] = W1m          # A: t10 (taps t9,t10,t11)
    W1ab[2:8, 64:128] = W1m        # B: t11 (taps t10,t11,t12=pad)

    W2m = np.zeros((128, 64), np.float32)
    W2m[:64, :] = w2[:, :, 0, 0].T
    W2m[64:, :] = w2[:, :, 0, 1].T

    b1s = np.concatenate([b1, b1]).reshape(128, 1).astype(np.float32)
    b2c = np.asarray(b2, np.float32).reshape(64, 1)
    gb1s = np.concatenate([gb1, gb1]).reshape(128, 1).astype(np.float32)
    gb2s = np.concatenate([gb2, gb2]).reshape(128, 1).astype(np.float32)
    gwd1 = np.concatenate([gw1, gw1], axis=0).astype(ml_dtypes.bfloat16)
    gwd2 = np.concatenate([gw2, gw2], axis=0).astype(ml_dtypes.bfloat16)
    wov = np.asarray(wo, np.float32)[0, :, 0, 0]
    wod2 = np.zeros((128, 2), np.float32)
    wod2[0:64, 0] = wov
    wod2[64:128, 1] = wov
    wod2 = wod2.astype(ml_dtypes.bfloat16)

    shared = {
        "W1ab": W1ab, "W2m": W2m, "b1s": b1s, "b2c": b2c,
        "gb1s": gb1s, "gb2s": gb2s, "gwd1": gwd1, "gwd2": gwd2, "wod2": wod2,
    }
    in_maps = []
    for k in range(NCORES):
        m = dict(shared)
        m["xt"] = xt_all[k]
        m["eidx"] = eidx_all[k]
        m["P"] = P_all[k]
        m["dsqk"] = dsqk_all[k]
        in_maps.append(m)
    return in_maps, slots, NGI, float(np.asarray(bo).reshape(-1)[0])


def _build(slots, NGI, bo_f):
    from concourse import bass, bacc, tile
    from concourse.masks import make_identity
    import mybir

    f32, bf16, i16 = mybir.dt.float32, mybir.dt.bfloat16, mybir.dt.int16
    f32r = mybir.dt.float32r
    TOT = len(slots)
    COLS = NGI * ICPG

    nc = bacc.Bacc("TRN2", target_bir_lowering=False, debug=False, num_devices=8,
                   dynamic_dma_scratch_size=65536)

    ext = {}
    for name, shape, dt in [
        ("xt", [8, NBC * 512], f32), ("W1ab", [8, 128], f32),
        ("W2m", [128, 64], f32), ("b1s", [128, 1], f32), ("b2c", [64, 1], f32),
        ("gb1s", [128, 1], f32), ("gb2s", [128, 1], f32),
        ("gwd1", [128, 64], bf16), ("gwd2", [128, 64], bf16),
        ("wod2", [128, 2], bf16), ("dsqk", [128, NBC], f32),
        ("eidx", [128, COLS], i16), ("P", [128, TOT * 128], bf16),
    ]:
        ext[name] = nc.dram_tensor(name, shape, dt, kind="ExternalInput").ap()
    y_ext = nc.dram_tensor("y", [128, 4 * NBC], f32, kind="ExternalOutput").ap()
    table0 = nc.dram_tensor("table0", [NSH, D], bf16).ap()
    table1 = nc.dram_tensor("table1", [NSH, D], bf16).ap()
    aggN = [nc.dram_tensor(f"aggN{L}", [NP, D], bf16).ap() for L in range(2)]
    aggS = [nc.dram_tensor(f"aggS{L}", [NSH, D], bf16).ap() for L in range(2)]

    with tile.TileContext(nc) as tc:
        with tc.tile_pool(name="const", bufs=1) as cp, \
             tc.tile_pool(name="hs", bufs=1) as hp:
            ct = {}
            for name in ("W1ab", "W2m", "b1s", "b2c", "gb1s", "gb2s",
                         "gwd1", "gwd2", "wod2", "dsqk", "eidx", "P"):
                t = cp.tile(list(ext[name].shape), ext[name].dtype, tag=name)
                nc.sync.dma_start(t[:], ext[name][:])
                ct[name] = t
            ident = cp.tile([128, 128], bf16, tag="ident")
            make_identity(nc, ident[:])
            y_nb = cp.tile([128, 4 * NBC], f32, tag="ynb")

            hs0 = hp.tile([128, NBC * D], bf16, tag="hs0")
            hs1 = hp.tile([128, NBC * D], bf16, tag="hs1")
            agg_sb0 = hp.tile([128, NBC * D], bf16, tag="asb0")
            agg_sb1 = hp.tile([128, NBC * D], bf16, tag="asb1")
            agg_sb = [agg_sb0, agg_sb1]

            # ---- conv stage: local table0 shard = dsq * relu(conv2(relu(conv1 x)))
            with tc.tile_pool(name="cv", bufs=3) as vp, \
                 tc.tile_pool(name="cvp", bufs=2, space="PSUM") as pp:
                xts = vp.tile([8, NBC * 512], f32, tag="xts")
                nc.sync.dma_start(xts[:], ext["xt"][:])
                for blk in range(NBC):
                    ph1 = pp.tile([128, 512], f32, tag="ph1", space="PSUM")
                    nc.tensor.matmul(ph1[:], lhsT=ct["W1ab"][:],
                                     rhs=xts[:, blk * 512:(blk + 1) * 512],
                                     start=True, stop=True)
                    h1 = vp.tile([128, 512], f32, tag="h1")
                    nc.scalar.activation(h1[:], ph1[:],
                                         mybir.ActivationFunctionType.Relu,
                                         bias=ct["b1s"][:, 0:1])
                    ph2 = pp.tile([64, 512], f32, tag="ph2", space="PSUM")
                    nc.tensor.matmul(ph2[:], lhsT=ct["W2m"][:], rhs=h1[:],
                                     start=True, stop=True)
                    h2 = vp.tile([64, 512], bf16, tag="h2")
                    nc.scalar.activation(h2[:], ph2[:],
                                         mybir.ActivationFunctionType.Relu,
                                         bias=ct["b2c"][:, 0:1])
                    for s in range(4):
                        ptp = pp.tile([128, 64], bf16, tag="ptp", space="PSUM")
                        nc.tensor.transpose(ptp[:], h2[:, s * 128:(s + 1) * 128],
                                            ident[0:64, 0:64])
                        nc.vector.tensor_scalar_mul(
                            hs0[:, blk * D + 64 * s: blk * D + 64 * (s + 1)],
                            ptp[:], ct["dsqk"][:, blk:blk + 1])
                nc.sync.dma_start(
                    table0.rearrange("(lb p) f -> p lb f", p=128), hs0[:])

            # ---- GCN layers
            for L in range(2):
                tbl = table0 if L == 0 else table1
                hs_cur = hs0 if L == 0 else hs1
                gwd = ct["gwd1"] if L == 0 else ct["gwd2"]
                gbs = ct["gb1s"] if L == 0 else ct["gb2s"]

                # scatter: partial aggregate over all NP dst rows
                # blocks grouped by 4: one PSUM tile [128, 4, 512] f32 --
                # each block's accumulator bank-aligned (matmul PSUM outputs
                # must start at a bank boundary); one copy + one DMA per group
                GB = 4
                with tc.tile_pool(name=f"g{L}", bufs=5) as gp, \
                     tc.tile_pool(name=f"st{L}", bufs=4) as sp, \
                     tc.tile_pool(name=f"sc{L}", bufs=3, space="PSUM") as qp:
                    g = None
                    pb4 = None
                    for i, (b, first, last) in enumerate(slots):
                        gi, j = divmod(i, SPG)
                        if j == 0:
                            g = gp.tile([128, SPG, D], bf16, tag="g")
                            nc.gpsimd.dma_gather(
                                g[:], tbl[:],
                                ct["eidx"][:, gi * ICPG:(gi + 1) * ICPG],
                                NIG, NIG, D)
                        bg, bo_ = divmod(b, GB)
                        if first and bo_ == 0:
                            pb4 = qp.tile([128, GB, D], f32, tag="pb4",
                                          space="PSUM")
                        nc.tensor.matmul(pb4[:, bo_, 0:D],
                                         lhsT=ct["P"][:, i * 128:(i + 1) * 128],
                                         rhs=g[:, j, :],
                                         start=first, stop=last)
                        if last and bo_ == GB - 1:
                            stg = sp.tile([128, GB * D], bf16, tag="stg")
                            if bg % 2 == 0:
                                nc.vector.tensor_copy(stg[:], pb4[:, :, 0:D])
                            else:
                                nc.scalar.activation(
                                    stg[:], pb4[:, :, 0:D],
                                    mybir.ActivationFunctionType.Copy)
                            nc.sync.dma_start(
                                aggN[L][bg * GB * 128:(bg + 1) * GB * 128, :]
                                .rearrange("(q p) f -> p q f", p=128),
                                stg[:])

                nc.gpsimd.collective_compute(
                    "ReduceScatter", mybir.AluOpType.add,
                    replica_groups=[list(range(NCORES))],
                    ins=[aggN[L][:]], outs=[aggS[L][:]])

                # finish: self-loop + dsq + W + bias + relu (local shard)
                with tc.tile_pool(name=f"fv{L}", bufs=3) as fv, \
                     tc.tile_pool(name=f"fp{L}", bufs=2, space="PSUM") as fp:
                    nc.sync.dma_start(
                        agg_sb[L][:],
                        aggS[L].rearrange("(lb p) f -> p lb f", p=128))
                    for lb in range(NBC):
                        ta = fv.tile([128, D], f32, tag="ta")
                        nc.vector.tensor_add(ta[:],
                                             agg_sb[L][:, lb * D:(lb + 1) * D],
                                             hs_cur[:, lb * D:(lb + 1) * D])
                        tsc = fv.tile([128, D], bf16, tag="tsc")
                        nc.vector.tensor_scalar_mul(tsc[:], ta[:],
                                                    ct["dsqk"][:, lb:lb + 1])
                        for pr in range(2):
                            tp = fp.tile([128, 128], bf16, tag="tp", space="PSUM")
                            nc.tensor.transpose(
                                tp[:], tsc[:, 128 * pr:128 * (pr + 1)], ident[:])
                            tps = fv.tile([128, 128], bf16, tag="tps")
                            if pr == 0:
                                nc.scalar.activation(
                                    tps[:], tp[:],
                                    mybir.ActivationFunctionType.Copy)
                            else:
                                nc.vector.tensor_copy(tps[:], tp[:])
                            wp = fp.tile([128, 128], f32, tag="wp", space="PSUM")
                            nc.tensor.matmul(wp[0:64, :], lhsT=gwd[0:64, :],
                                             rhs=tps[0:64, :], start=True, stop=True)
                            nc.tensor.matmul(wp[64:128, :], lhsT=gwd[64:128, :],
                                             rhs=tps[64:128, :], start=True, stop=True)
                            if L == 0:
                                hn = fv.tile([128, 128], bf16, tag="hn")
                                nc.scalar.activation(
                                    hn[:], wp[:],
                                    mybir.ActivationFunctionType.Relu,
                                    bias=gbs[:, 0:1])
                                tb = fp.tile([128, 128], bf16, tag="tb",
                                             space="PSUM")
                                nc.tensor.transpose(tb[:], hn[:], ident[:])
                                nc.vector.tensor_scalar_mul(
                                    hs1[:, lb * D + 128 * pr:
                                        lb * D + 128 * (pr + 1)],
                                    tb[:], ct["dsqk"][:, lb:lb + 1])
                            else:
                                h4 = fv.tile([128, 128], bf16, tag="h4")
                                nc.scalar.activation(
                                    h4[:], wp[:],
                                    mybir.ActivationFunctionType.Relu,
                                    bias=gbs[:, 0:1])
                                yp = fp.tile([128, 2], f32, tag="yp",
                                             space="PSUM")
                                nc.tensor.matmul(yp[:], lhsT=h4[:],
                                                 rhs=ct["wod2"][:],
                                                 start=True, stop=True)
                                nc.vector.tensor_scalar_add(
                                    y_nb[:, lb * 4 + pr * 2: lb * 4 + pr * 2 + 2],
                                    yp[:], bo_f)
                    if L == 0:
                        nc.sync.dma_start(
                            table1.rearrange("(lb p) f -> p lb f", p=128),
                            hs1[:])
            nc.sync.dma_start(y_ext[:], y_nb[:])
    nc.compile()
    return nc


def _run(inputs):
    from concourse.bass_utils import run_bass_kernel_spmd

    in_maps, slots, NGI, bo_f = _host_prep(
        inputs["x"], inputs["edge_index"], inputs["w1"], inputs["b1"],
        inputs["w2"], inputs["b2"], inputs["gw1"], inputs["gb1"],
        inputs["gw2"], inputs["gb2"], inputs["wo"], inputs["bo"])

    key = (hash(tuple(slots)), NGI)
    if key not in _cache:
        _cache[key] = _build(slots, NGI, bo_f)
    nc = _cache[key]

    res = run_bass_kernel_spmd(nc, in_maps, list(range(8)))
    y = np.zeros((B, N), dtype=np.float32)
    for k in range(NCORES):
        y_nb = res.results[k]["y"]          # [128, 4*NBC]
        for lb in range(NBC):
            lo = k * NSH + lb * 128
            hi = min(lo + 128, N)
            if hi <= lo:
                continue
            for s in range(B):
                y[s, lo:hi] = y_nb[: hi - lo, lb * 4 + s]
    return y


def kernel(**inputs):
    return _run(inputs)


# revision 24
# speedup vs baseline: 1.2922x; 1.0428x over previous
"""GraphWaveNet kernel for Trainium2 (Bass/Tile), 8 NeuronCores.

v2: edge sharding by SOURCE block + ReduceScatter (no AllGathers).

Algorithm notes:
- Only t=11 survives the final 1x1 conv; GCN doesn't mix time. So conv
  stack evaluated at t in {10,11} only, GCN on B=4 slices (D=256 cols =
  4 slices x 64 feats).
- GCN: with Hs = dsq*h, agg_n = dsq_n * (sum_{e->n} Hs[src_e] + Hs[n]).
- Sharding: core k owns node rows [1280k, 1280(k+1)). Conv computes the
  local shard of the Hs table (no collective). Edges are assigned to the
  core owning their SRC: gathers hit only the local table shard.
  Scatter (one-hot P matmuls into per-dst-block PSUM) produces a partial
  aggregate over ALL 10240 nodes; one f32 ReduceScatter per layer sums
  partials and hands each core its own 1280-row slice. Self-loop + W +
  bias + relu are then local.
- SPMD uniformity: slot schedule per dst block = max over cores of
  ceil(edges/128) (program identical across cores; per-core eidx/P data
  differ, padded with index-0 gathers and zero P columns).
- Gathers: 16 slots (2048 edges) per indirect DMA to amortize SWDGE
  descriptor-gen overhead (~1us/instruction on the Pool engine).
"""

import sys

sys.path.insert(0, "/opt/trn_rl_repo")

import numpy as np
import ml_dtypes

B, T, N, FIN, H, E = 4, 12, 10000, 2, 64, 80000
NCORES = 8
NB80 = 80                 # dst blocks of 128 nodes
NP = NB80 * 128           # padded node count (10240)
NSH = NP // NCORES        # node rows per core (1280)
NBC = NB80 // NCORES      # node blocks per core (10)
D = 4 * H                 # 256 = 4 slices x 64 feats
SPG = 8                   # slots (of 128 edges) per dma_gather (1024 idxs)
NIG = SPG * 128           # indices per gather instruction
ICPG = NIG // 16          # idx tile columns per gather instruction (64)

_cache = {}


def _host_prep(x, edge_index, w1, b1, w2, b2, gw1, gb1, gw2, gb2, wo, bo):
    x = np.asarray(x, np.float32)
    src = np.asarray(edge_index[0]).astype(np.int64)
    dst = np.asarray(edge_index[1]).astype(np.int64)

    deg = np.bincount(dst, minlength=N).astype(np.float64) + 1.0
    dsq = (deg ** -0.5).astype(np.float32)
    dsq_pad = np.ones(NP, dtype=np.float32)
    dsq_pad[:N] = dsq

    # ---- per-core edge partition by src owner, dst-sorted
    owner = src // NSH
    es_k, ed_k, cnt = [], [], np.zeros((NCORES, NB80), np.int64)
    for k in range(NCORES):
        m = owner == k
        es, ed = src[m], dst[m]
        o = np.argsort(ed, kind="stable")
        es, ed = es[o], ed[o]
        es_k.append(es)
        ed_k.append(ed)
        cnt[k] = np.bincount(ed // 128, minlength=NB80)

    S_b = np.maximum(1, (cnt + 127) // 128).max(axis=0)   # slots per block
    slots = []                                            # (block, first, last)
    for b in range(NB80):
        for j in range(int(S_b[b])):
            slots.append((b, j == 0, j == int(S_b[b]) - 1))
    TOT = len(slots)
    NGI = (TOT + SPG - 1) // SPG

    # eidx layout (dma_gather ucode contract, queue 0): within gather
    # instruction gi, flat index j in [0, NIG) lives at SBUF position
    # [16 + (j % 16), gi * ICPG + j // 16]; edge j lands at out[j%128, j//128].
    eidx_all = np.zeros((NCORES, 128, NGI * ICPG), np.int16)
    P_all = np.zeros((NCORES, 128, TOT * 128), np.float32)
    for k in range(NCORES):
        es, ed = es_k[k], ed_k[k]
        bounds = np.searchsorted(ed, np.arange(NB80 + 1) * 128)
        si = 0
        for b in range(NB80):
            e0, e1 = int(bounds[b]), int(bounds[b + 1])
            for j in range(int(S_b[b])):
                a = e0 + j * 128
                bb = min(a + 128, e1)
                ne = max(0, bb - a)
                if ne:
                    gi, c = divmod(si, SPG)
                    jj = c * 128 + np.arange(ne)        # flat idx within instr
                    eidx_all[k, 16 + (jj % 16), gi * ICPG + jj // 16] = \
                        (es[a:bb] - k * NSH).astype(np.int16)
                    P_all[k, np.arange(ne), si * 128 + (ed[a:bb] - b * 128)] = 1.0
                si += 1
    P_all = P_all.astype(ml_dtypes.bfloat16)

    # ---- conv input: per block 8 rows (t,c) for t in {9,10,11} + 2 zero rows,
    # cols = 4 slices x 128 nodes
    xpad = np.zeros((B, 3, FIN, NP), np.float32)
    xpad[:, :, :, :N] = x[:, 9:12, :, :].transpose(0, 1, 3, 2)  # [s, ti, c, n]
    xv = xpad.reshape(B, 6, NCORES, NBC, 128)                   # [s, row, k, blk, p]
    xt_all = np.zeros((NCORES, 8, NBC * 4 * 128), np.float32)
    xt_all[:, :6] = xv.transpose(2, 1, 3, 0, 4).reshape(NCORES, 6, NBC * 4 * 128)
    xt_all = xt_all.astype(ml_dtypes.bfloat16)

    dsqk_all = dsq_pad.reshape(NCORES, NBC, 128).transpose(0, 2, 1).copy()

    # ---- weights
    W1m = np.zeros((6, 64), np.float32)
    for kk in range(3):
        for c in range(FIN):
            W1m[2 * kk + c, :] = w1[:, c, 0, kk]
    W1ab = np.zeros((8, 128), np.float32)
    W1ab[0:6, 0:64] = W1m          # A: t10 (taps t9,t10,t11)
    W1ab[2:8, 64:128] = W1m        # B: t11 (taps t10,t11,t12=pad)
    W1ab = W1ab.astype(ml_dtypes.bfloat16)

    W2m = np.zeros((128, 64), np.float32)
    W2m[:64, :] = w2[:, :, 0, 0].T
    W2m[64:, :] = w2[:, :, 0, 1].T
    W2m = W2m.astype(ml_dtypes.bfloat16)

    b1s = np.concatenate([b1, b1]).reshape(128, 1).astype(np.float32)
    b2c = np.asarray(b2, np.float32).reshape(64, 1)
    gb1s = np.concatenate([gb1, gb1]).reshape(128, 1).astype(np.float32)
    gb2s = np.concatenate([gb2, gb2]).reshape(128, 1).astype(np.float32)
    gwd1 = np.concatenate([gw1, gw1], axis=0).astype(ml_dtypes.bfloat16)
    gwd2 = np.concatenate([gw2, gw2], axis=0).astype(ml_dtypes.bfloat16)
    wov = np.asarray(wo, np.float32)[0, :, 0, 0]
    wod2 = np.zeros((128, 2), np.float32)
    wod2[0:64, 0] = wov
    wod2[64:128, 1] = wov
    wod2 = wod2.astype(ml_dtypes.bfloat16)

    shared = {
        "W1ab": W1ab, "W2m": W2m, "b1s": b1s, "b2c": b2c,
        "gb1s": gb1s, "gb2s": gb2s, "gwd1": gwd1, "gwd2": gwd2, "wod2": wod2,
    }
    in_maps = []
    for k in range(NCORES):
        m = dict(shared)
        m["xt"] = xt_all[k]
        m["eidx"] = eidx_all[k]
        m["P"] = P_all[k]
        m["dsqk"] = dsqk_all[k]
        in_maps.append(m)
    return in_maps, slots, NGI, float(np.asarray(bo).reshape(-1)[0])


def _build(slots, NGI, bo_f):
    from concourse import bass, bacc, tile
    from concourse.masks import make_identity
    import mybir

    f32, bf16, i16 = mybir.dt.float32, mybir.dt.bfloat16, mybir.dt.int16
    f32r = mybir.dt.float32r
    TOT = len(slots)
    COLS = NGI * ICPG

    nc = bacc.Bacc("TRN2", target_bir_lowering=False, debug=False, num_devices=8,
                   dynamic_dma_scratch_size=65536)

    ext = {}
    for name, shape, dt in [
        ("xt", [8, NBC * 512], bf16), ("W1ab", [8, 128], bf16),
        ("W2m", [128, 64], bf16), ("b1s", [128, 1], f32), ("b2c", [64, 1], f32),
        ("gb1s", [128, 1], f32), ("gb2s", [128, 1], f32),
        ("gwd1", [128, 64], bf16), ("gwd2", [128, 64], bf16),
        ("wod2", [128, 2], bf16), ("dsqk", [128, NBC], f32),
        ("eidx", [128, COLS], i16), ("P", [128, TOT * 128], bf16),
    ]:
        ext[name] = nc.dram_tensor(name, shape, dt, kind="ExternalInput").ap()
    y_ext = nc.dram_tensor("y", [128, 4 * NBC], f32, kind="ExternalOutput").ap()
    table0 = nc.dram_tensor("table0", [NSH, D], bf16).ap()
    table1 = nc.dram_tensor("table1", [NSH, D], bf16).ap()
    aggN = [nc.dram_tensor(f"aggN{L}", [NP, D], bf16).ap() for L in range(2)]
    aggS = [nc.dram_tensor(f"aggS{L}", [NSH, D], bf16).ap() for L in range(2)]

    with tile.TileContext(nc) as tc:
        with tc.tile_pool(name="const", bufs=1) as cp, \
             tc.tile_pool(name="hs", bufs=1) as hp:
            ct = {}
            for name in ("W1ab", "W2m", "b1s", "b2c", "dsqk", "gb1s",
                         "gb2s", "gwd1", "gwd2", "wod2", "eidx", "P"):
                t = cp.tile(list(ext[name].shape), ext[name].dtype, tag=name)
                nc.sync.dma_start(t[:], ext[name][:])
                ct[name] = t
            ident = cp.tile([128, 128], bf16, tag="ident")
            make_identity(nc, ident[:])
            y_nb = cp.tile([128, 4 * NBC], f32, tag="ynb")

            hs0 = hp.tile([128, NBC * D], bf16, tag="hs0")
            hs1 = hp.tile([128, NBC * D], bf16, tag="hs1")
            agg_sb0 = hp.tile([128, NBC * D], bf16, tag="asb0")
            agg_sb1 = hp.tile([128, NBC * D], bf16, tag="asb1")
            agg_sb = [agg_sb0, agg_sb1]

            # ---- conv stage: local table0 shard = dsq * relu(conv2(relu(conv1 x)))
            with tc.tile_pool(name="cv", bufs=3) as vp, \
                 tc.tile_pool(name="cvp", bufs=2, space="PSUM") as pp:
                xts = vp.tile([8, NBC * 512], bf16, tag="xts")
                nc.sync.dma_start(xts[:], ext["xt"][:])
                for blk in range(NBC):
                    ph1 = pp.tile([128, 512], f32, tag="ph1", space="PSUM")
                    nc.tensor.matmul(ph1[:], lhsT=ct["W1ab"][:],
                                     rhs=xts[:, blk * 512:(blk + 1) * 512],
                                     start=True, stop=True)
                    h1 = vp.tile([128, 512], bf16, tag="h1")
                    nc.scalar.activation(h1[:], ph1[:],
                                         mybir.ActivationFunctionType.Relu,
                                         bias=ct["b1s"][:, 0:1])
                    ph2 = pp.tile([64, 512], f32, tag="ph2", space="PSUM")
                    nc.tensor.matmul(ph2[:], lhsT=ct["W2m"][:], rhs=h1[:],
                                     start=True, stop=True)
                    h2 = vp.tile([64, 512], bf16, tag="h2")
                    nc.scalar.activation(h2[:], ph2[:],
                                         mybir.ActivationFunctionType.Relu,
                                         bias=ct["b2c"][:, 0:1])
                    for s in range(4):
                        ptp = pp.tile([128, 64], bf16, tag="ptp", space="PSUM")
                        nc.tensor.transpose(ptp[:], h2[:, s * 128:(s + 1) * 128],
                                            ident[0:64, 0:64])
                        nc.vector.tensor_scalar_mul(
                            hs0[:, blk * D + 64 * s: blk * D + 64 * (s + 1)],
                            ptp[:], ct["dsqk"][:, blk:blk + 1])
                nc.sync.dma_start(
                    table0.rearrange("(lb p) f -> p lb f", p=128), hs0[:])

            # ---- GCN layers
            for L in range(2):
                tbl = table0 if L == 0 else table1
                hs_cur = hs0 if L == 0 else hs1
                gwd = ct["gwd1"] if L == 0 else ct["gwd2"]
                gbs = ct["gb1s"] if L == 0 else ct["gb2s"]

                # scatter: partial aggregate over all NP dst rows
                # blocks grouped by 4: one PSUM tile [128, 4, 512] f32 --
                # each block's accumulator bank-aligned (matmul PSUM outputs
                # must start at a bank boundary); one copy + one DMA per group
                GB = 4
                with tc.tile_pool(name=f"g{L}", bufs=5) as gp, \
                     tc.tile_pool(name=f"st{L}", bufs=4) as sp, \
                     tc.tile_pool(name=f"sc{L}", bufs=3, space="PSUM") as qp:
                    g = None
                    pb4 = None
                    for i, (b, first, last) in enumerate(slots):
                        gi, j = divmod(i, SPG)
                        if j == 0:
                            g = gp.tile([128, SPG, D], bf16, tag="g")
                            nc.gpsimd.dma_gather(
                                g[:], tbl[:],
                                ct["eidx"][:, gi * ICPG:(gi + 1) * ICPG],
                                NIG, NIG, D)
                        bg, bo_ = divmod(b, GB)
                        if first and bo_ == 0:
                            pb4 = qp.tile([128, GB, D], f32, tag="pb4",
                                          space="PSUM")
                        nc.tensor.matmul(pb4[:, bo_, 0:D],
                                         lhsT=ct["P"][:, i * 128:(i + 1) * 128],
                                         rhs=g[:, j, :],
                                         start=first, stop=last)
                        if last and bo_ == GB - 1:
                            stg = sp.tile([128, GB * D], bf16, tag="stg")
                            if bg % 2 == 0:
                                nc.vector.tensor_copy(stg[:], pb4[:, :, 0:D])
                            else:
                                nc.scalar.activation(
                                    stg[:], pb4[:, :, 0:D],
                                    mybir.ActivationFunctionType.Copy)
                            nc.sync.dma_start(
                                aggN[L][bg * GB * 128:(bg + 1) * GB * 128, :]
                                .rearrange("(q p) f -> p q f", p=128),
                                stg[:])

                nc.gpsimd.collective_compute(
                    "ReduceScatter", mybir.AluOpType.add,
                    replica_groups=[list(range(NCORES))],
                    ins=[aggN[L][:]], outs=[aggS[L][:]])

                # finish: self-loop + dsq + W + bias + relu (local shard)
                with tc.tile_pool(name=f"fv{L}", bufs=4) as fv, \
                     tc.tile_pool(name=f"fp{L}", bufs=2, space="PSUM") as fp:
                    nc.sync.dma_start(
                        agg_sb[L][:],
                        aggS[L].rearrange("(lb p) f -> p lb f", p=128))
                    for lb in range(NBC):
                        ta = fv.tile([128, D], f32, tag="ta")
                        nc.vector.tensor_add(ta[:],
                                             agg_sb[L][:, lb * D:(lb + 1) * D],
                                             hs_cur[:, lb * D:(lb + 1) * D])
                        tsc = fv.tile([128, D], bf16, tag="tsc")
                        nc.scalar.activation(tsc[:], ta[:],
                                             mybir.ActivationFunctionType.Copy,
                                             scale=ct["dsqk"][:, lb:lb + 1])
                        for pr in range(2):
                            tp = fp.tile([128, 128], bf16, tag="tp", space="PSUM")
                            nc.tensor.transpose(
                                tp[:], tsc[:, 128 * pr:128 * (pr + 1)], ident[:])
                            tps = fv.tile([128, 128], bf16, tag="tps")
                            nc.vector.tensor_copy(tps[:], tp[:])
                            wp = fp.tile([128, 128], f32, tag="wp", space="PSUM")
                            nc.tensor.matmul(wp[0:64, :], lhsT=gwd[0:64, :],
                                             rhs=tps[0:64, :], start=True, stop=True)
                            nc.tensor.matmul(wp[64:128, :], lhsT=gwd[64:128, :],
                                             rhs=tps[64:128, :], start=True, stop=True)
                            if L == 0:
                                hn = fv.tile([128, 128], bf16, tag="hn")
                                nc.scalar.activation(
                                    hn[:], wp[:],
                                    mybir.ActivationFunctionType.Relu,
                                    bias=gbs[:, 0:1])
                                tb = fp.tile([128, 128], bf16, tag="tb",
                                             space="PSUM")
                                nc.tensor.transpose(tb[:], hn[:], ident[:])
                                nc.vector.tensor_scalar_mul(
                                    hs1[:, lb * D + 128 * pr:
                                        lb * D + 128 * (pr + 1)],
                                    tb[:], ct["dsqk"][:, lb:lb + 1])
                            else:
                                h4 = fv.tile([128, 128], bf16, tag="h4")
                                nc.scalar.activation(
                                    h4[:], wp[:],
                                    mybir.ActivationFunctionType.Relu,
                                    bias=gbs[:, 0:1])
                                yp = fp.tile([128, 2], f32, tag="yp",
                                             space="PSUM")
                                nc.tensor.matmul(yp[:], lhsT=h4[:],
                                                 rhs=ct["wod2"][:],
                                                 start=True, stop=True)
                                nc.vector.tensor_scalar_add(
                                    y_nb[:, lb * 4 + pr * 2: lb * 4 + pr * 2 + 2],
                                    yp[:], bo_f)
                    if L == 0:
                        nc.sync.dma_start(
                            table1.rearrange("(lb p) f -> p lb f", p=128),
                            hs1[:])
            nc.sync.dma_start(y_ext[:], y_nb[:])
    nc.compile()
    return nc


def _run(inputs):
    from concourse.bass_utils import run_bass_kernel_spmd

    in_maps, slots, NGI, bo_f = _host_prep(
        inputs["x"], inputs["edge_index"], inputs["w1"], inputs["b1"],
        inputs["w2"], inputs["b2"], inputs["gw1"], inputs["gb1"],
        inputs["gw2"], inputs["gb2"], inputs["wo"], inputs["bo"])

    key = (hash(tuple(slots)), NGI)
    if key not in _cache:
        _cache[key] = _build(slots, NGI, bo_f)
    nc = _cache[key]

    res = run_bass_kernel_spmd(nc, in_maps, list(range(8)))
    y = np.zeros((B, N), dtype=np.float32)
    for k in range(NCORES):
        y_nb = res.results[k]["y"]          # [128, 4*NBC]
        for lb in range(NBC):
            lo = k * NSH + lb * 128
            hi = min(lo + 128, N)
            if hi <= lo:
                continue
            for s in range(B):
                y[s, lo:hi] = y_nb[: hi - lo, lb * 4 + s]
    return y


def kernel(**inputs):
    return _run(inputs)


# revision 25
# speedup vs baseline: 1.3623x; 1.0543x over previous
"""GraphWaveNet kernel for Trainium2 (Bass/Tile), 8 NeuronCores.

v2: edge sharding by SOURCE block + ReduceScatter (no AllGathers).

Algorithm notes:
- Only t=11 survives the final 1x1 conv; GCN doesn't mix time. So conv
  stack evaluated at t in {10,11} only, GCN on B=4 slices (D=256 cols =
  4 slices x 64 feats).
- GCN: with Hs = dsq*h, agg_n = dsq_n * (sum_{e->n} Hs[src_e] + Hs[n]).
- Sharding: core k owns node rows [1280k, 1280(k+1)). Conv computes the
  local shard of the Hs table (no collective). Edges are assigned to the
  core owning their SRC: gathers hit only the local table shard.
  Scatter (one-hot P matmuls into per-dst-block PSUM) produces a partial
  aggregate over ALL 10240 nodes; one f32 ReduceScatter per layer sums
  partials and hands each core its own 1280-row slice. Self-loop + W +
  bias + relu are then local.
- SPMD uniformity: slot schedule per dst block = max over cores of
  ceil(edges/128) (program identical across cores; per-core eidx/P data
  differ, padded with index-0 gathers and zero P columns).
- Gathers: 16 slots (2048 edges) per indirect DMA to amortize SWDGE
  descriptor-gen overhead (~1us/instruction on the Pool engine).
"""

import sys

sys.path.insert(0, "/opt/trn_rl_repo")

import numpy as np
import ml_dtypes

B, T, N, FIN, H, E = 4, 12, 10000, 2, 64, 80000
NCORES = 8
NB80 = 80                 # dst blocks of 128 nodes
NP = NB80 * 128           # padded node count (10240)
NSH = NP // NCORES        # node rows per core (1280)
NBC = NB80 // NCORES      # node blocks per core (10)
D = 4 * H                 # 256 = 4 slices x 64 feats
SPG = 8                   # slots (of 128 edges) per dma_gather (1024 idxs)
NIG = SPG * 128           # indices per gather instruction
ICPG = NIG // 16          # idx tile columns per gather instruction (64)

_cache = {}


def _host_prep(x, edge_index, w1, b1, w2, b2, gw1, gb1, gw2, gb2, wo, bo):
    x = np.asarray(x, np.float32)
    src = np.asarray(edge_index[0]).astype(np.int64)
    dst = np.asarray(edge_index[1]).astype(np.int64)

    deg = np.bincount(dst, minlength=N).astype(np.float64) + 1.0
    dsq = (deg ** -0.5).astype(np.float32)
    dsq_pad = np.ones(NP, dtype=np.float32)
    dsq_pad[:N] = dsq

    # ---- per-core edge partition by src owner, dst-sorted
    owner = src // NSH
    es_k, ed_k, cnt = [], [], np.zeros((NCORES, NB80), np.int64)
    for k in range(NCORES):
        m = owner == k
        es, ed = src[m], dst[m]
        o = np.argsort(ed, kind="stable")
        es, ed = es[o], ed[o]
        es_k.append(es)
        ed_k.append(ed)
        cnt[k] = np.bincount(ed // 128, minlength=NB80)

    S_b = np.maximum(1, (cnt + 127) // 128).max(axis=0)   # slots per block
    slots = []                                            # (block, first, last)
    for b in range(NB80):
        for j in range(int(S_b[b])):
            slots.append((b, j == 0, j == int(S_b[b]) - 1))
    TOT = len(slots)
    NGI = (TOT + SPG - 1) // SPG

    # eidx layout (dma_gather ucode contract, queue 0): within gather
    # instruction gi, flat index j in [0, NIG) lives at SBUF position
    # [16 + (j % 16), gi * ICPG + j // 16]; edge j lands at out[j%128, j//128].
    eidx_all = np.zeros((NCORES, 128, NGI * ICPG), np.int16)
    P_all = np.zeros((NCORES, 128, TOT * 128), np.float32)
    for k in range(NCORES):
        es, ed = es_k[k], ed_k[k]
        bounds = np.searchsorted(ed, np.arange(NB80 + 1) * 128)
        si = 0
        for b in range(NB80):
            e0, e1 = int(bounds[b]), int(bounds[b + 1])
            for j in range(int(S_b[b])):
                a = e0 + j * 128
                bb = min(a + 128, e1)
                ne = max(0, bb - a)
                if ne:
                    gi, c = divmod(si, SPG)
                    jj = c * 128 + np.arange(ne)        # flat idx within instr
                    eidx_all[k, 16 + (jj % 16), gi * ICPG + jj // 16] = \
                        (es[a:bb] - k * NSH).astype(np.int16)
                    P_all[k, np.arange(ne), si * 128 + (ed[a:bb] - b * 128)] = 1.0
                si += 1
    P_all = P_all.astype(ml_dtypes.bfloat16)

    # ---- conv input: per block 8 rows (t,c) for t in {9,10,11} + 2 zero rows,
    # cols = 4 slices x 128 nodes
    xpad = np.zeros((B, 3, FIN, NP), np.float32)
    xpad[:, :, :, :N] = x[:, 9:12, :, :].transpose(0, 1, 3, 2)  # [s, ti, c, n]
    xv = xpad.reshape(B, 6, NCORES, NBC, 128)                   # [s, row, k, blk, p]
    xt_all = np.zeros((NCORES, 8, NBC * 4 * 128), np.float32)
    xt_all[:, :6] = xv.transpose(2, 1, 3, 0, 4).reshape(NCORES, 6, NBC * 4 * 128)
    xt_all = xt_all.astype(ml_dtypes.bfloat16)

    dsqk_all = dsq_pad.reshape(NCORES, NBC, 128).transpose(0, 2, 1).copy()

    # ---- weights
    W1m = np.zeros((6, 64), np.float32)
    for kk in range(3):
        for c in range(FIN):
            W1m[2 * kk + c, :] = w1[:, c, 0, kk]
    W1ab = np.zeros((8, 128), np.float32)
    W1ab[0:6, 0:64] = W1m          # A: t10 (taps t9,t10,t11)
    W1ab[2:8, 64:128] = W1m        # B: t11 (taps t10,t11,t12=pad)
    W1ab = W1ab.astype(ml_dtypes.bfloat16)

    W2m = np.zeros((128, 64), np.float32)
    W2m[:64, :] = w2[:, :, 0, 0].T
    W2m[64:, :] = w2[:, :, 0, 1].T
    W2m = W2m.astype(ml_dtypes.bfloat16)

    b1s = np.concatenate([b1, b1]).reshape(128, 1).astype(np.float32)
    b2c = np.asarray(b2, np.float32).reshape(64, 1)
    gb1s = np.concatenate([gb1, gb1]).reshape(128, 1).astype(np.float32)
    gb2s = np.concatenate([gb2, gb2]).reshape(128, 1).astype(np.float32)
    gwd1 = np.concatenate([gw1, gw1], axis=0).astype(ml_dtypes.bfloat16)
    gwd2 = np.concatenate([gw2, gw2], axis=0).astype(ml_dtypes.bfloat16)
    wov = np.asarray(wo, np.float32)[0, :, 0, 0]
    wod2 = np.zeros((128, 2), np.float32)
    wod2[0:64, 0] = wov
    wod2[64:128, 1] = wov
    wod2 = wod2.astype(ml_dtypes.bfloat16)

    shared = {
        "W1ab": W1ab, "W2m": W2m, "b1s": b1s, "b2c": b2c,
        "gb1s": gb1s, "gb2s": gb2s, "gwd1": gwd1, "gwd2": gwd2, "wod2": wod2,
    }
    in_maps = []
    for k in range(NCORES):
        m = dict(shared)
        m["xt"] = xt_all[k]
        m["eidx"] = eidx_all[k]
        m["P"] = P_all[k]
        m["dsqk"] = dsqk_all[k]
        in_maps.append(m)
    return in_maps, slots, NGI, float(np.asarray(bo).reshape(-1)[0])


def _build(slots, NGI, bo_f):
    from concourse import bass, bacc, tile
    from concourse.masks import make_identity
    import mybir

    f32, bf16, i16 = mybir.dt.float32, mybir.dt.bfloat16, mybir.dt.int16
    f32r = mybir.dt.float32r
    TOT = len(slots)
    COLS = NGI * ICPG

    nc = bacc.Bacc("TRN2", target_bir_lowering=False, debug=False, num_devices=8,
                   dynamic_dma_scratch_size=65536)

    ext = {}
    for name, shape, dt in [
        ("xt", [8, NBC * 512], bf16), ("W1ab", [8, 128], bf16),
        ("W2m", [128, 64], bf16), ("b1s", [128, 1], f32), ("b2c", [64, 1], f32),
        ("gb1s", [128, 1], f32), ("gb2s", [128, 1], f32),
        ("gwd1", [128, 64], bf16), ("gwd2", [128, 64], bf16),
        ("wod2", [128, 2], bf16), ("dsqk", [128, NBC], f32),
        ("eidx", [128, COLS], i16), ("P", [128, TOT * 128], bf16),
    ]:
        ext[name] = nc.dram_tensor(name, shape, dt, kind="ExternalInput").ap()
    y_ext = nc.dram_tensor("y", [128, 4 * NBC], f32, kind="ExternalOutput").ap()
    table0 = nc.dram_tensor("table0", [NSH, D], bf16).ap()
    table1 = nc.dram_tensor("table1", [NSH, D], bf16).ap()
    aggN = [nc.dram_tensor(f"aggN{L}", [NP, D], bf16).ap() for L in range(2)]
    aggS = [nc.dram_tensor(f"aggS{L}", [NSH, D], bf16).ap() for L in range(2)]

    with tile.TileContext(nc) as tc:
        with tc.tile_pool(name="const", bufs=1) as cp, \
             tc.tile_pool(name="hs", bufs=1) as hp:
            ct = {}
            for name in ("W1ab", "W2m", "b1s", "b2c", "dsqk", "gb1s",
                         "gb2s", "gwd1", "gwd2", "wod2", "eidx"):
                t = cp.tile(list(ext[name].shape), ext[name].dtype, tag=name)
                nc.sync.dma_start(t[:], ext[name][:])
                ct[name] = t
            # P (5MB) loaded after conv's input DMA is issued -- SP runs its
            # queue in order and P would otherwise delay the conv start; P is
            # first needed by the scatter matmuls ~45us in.
            Pt = cp.tile(list(ext["P"].shape), ext["P"].dtype, tag="P")
            ct["P"] = Pt
            ident = cp.tile([128, 128], bf16, tag="ident")
            make_identity(nc, ident[:])
            y_nb = cp.tile([128, 4 * NBC], f32, tag="ynb")

            hs0 = hp.tile([128, NBC * D], bf16, tag="hs0")
            hs1 = hp.tile([128, NBC * D], bf16, tag="hs1")
            agg_sb0 = hp.tile([128, NBC * D], bf16, tag="asb0")
            agg_sb1 = hp.tile([128, NBC * D], bf16, tag="asb1")
            agg_sb = [agg_sb0, agg_sb1]

            # ---- conv stage: local table0 shard = dsq * relu(conv2(relu(conv1 x)))
            with tc.tile_pool(name="cv", bufs=3) as vp, \
                 tc.tile_pool(name="cvp", bufs=2, space="PSUM") as pp:
                xts = vp.tile([8, NBC * 512], bf16, tag="xts")
                nc.sync.dma_start(xts[:], ext["xt"][:])
                nc.sync.dma_start(Pt[:], ext["P"][:])
                for blk in range(NBC):
                    ph1 = pp.tile([128, 512], f32, tag="ph1", space="PSUM")
                    nc.tensor.matmul(ph1[:], lhsT=ct["W1ab"][:],
                                     rhs=xts[:, blk * 512:(blk + 1) * 512],
                                     start=True, stop=True)
                    h1 = vp.tile([128, 512], bf16, tag="h1")
                    nc.vector.tensor_scalar(h1[:], ph1[:],
                                            ct["b1s"][:, 0:1], 0.0,
                                            mybir.AluOpType.add,
                                            mybir.AluOpType.max)
                    ph2 = pp.tile([64, 512], f32, tag="ph2", space="PSUM")
                    nc.tensor.matmul(ph2[:], lhsT=ct["W2m"][:], rhs=h1[:],
                                     start=True, stop=True)
                    h2 = vp.tile([64, 512], bf16, tag="h2")
                    nc.scalar.activation(h2[:], ph2[:],
                                         mybir.ActivationFunctionType.Relu,
                                         bias=ct["b2c"][:, 0:1])
                    for s in range(4):
                        ptp = pp.tile([128, 64], bf16, tag="ptp", space="PSUM")
                        nc.tensor.transpose(ptp[:], h2[:, s * 128:(s + 1) * 128],
                                            ident[0:64, 0:64])
                        nc.vector.tensor_scalar_mul(
                            hs0[:, blk * D + 64 * s: blk * D + 64 * (s + 1)],
                            ptp[:], ct["dsqk"][:, blk:blk + 1])
                nc.sync.dma_start(
                    table0.rearrange("(lb p) f -> p lb f", p=128), hs0[:])

            # ---- GCN layers
            for L in range(2):
                tbl = table0 if L == 0 else table1
                hs_cur = hs0 if L == 0 else hs1
                gwd = ct["gwd1"] if L == 0 else ct["gwd2"]
                gbs = ct["gb1s"] if L == 0 else ct["gb2s"]

                # scatter: partial aggregate over all NP dst rows
                # blocks grouped by 4: one PSUM tile [128, 4, 512] f32 --
                # each block's accumulator bank-aligned (matmul PSUM outputs
                # must start at a bank boundary); one copy + one DMA per group
                GB = 4
                with tc.tile_pool(name=f"g{L}", bufs=5) as gp, \
                     tc.tile_pool(name=f"st{L}", bufs=4) as sp, \
                     tc.tile_pool(name=f"sc{L}", bufs=3, space="PSUM") as qp:
                    g = None
                    pb4 = None
                    for i, (b, first, last) in enumerate(slots):
                        gi, j = divmod(i, SPG)
                        if j == 0:
                            g = gp.tile([128, SPG, D], bf16, tag="g")
                            nc.gpsimd.dma_gather(
                                g[:], tbl[:],
                                ct["eidx"][:, gi * ICPG:(gi + 1) * ICPG],
                                NIG, NIG, D)
                        bg, bo_ = divmod(b, GB)
                        if first and bo_ == 0:
                            pb4 = qp.tile([128, GB, D], f32, tag="pb4",
                                          space="PSUM")
                        nc.tensor.matmul(pb4[:, bo_, 0:D],
                                         lhsT=ct["P"][:, i * 128:(i + 1) * 128],
                                         rhs=g[:, j, :],
                                         start=first, stop=last)
                        if last and bo_ == GB - 1:
                            stg = sp.tile([128, GB * D], bf16, tag="stg")
                            if bg % 2 == 0:
                                nc.vector.tensor_copy(stg[:], pb4[:, :, 0:D])
                            else:
                                nc.scalar.activation(
                                    stg[:], pb4[:, :, 0:D],
                                    mybir.ActivationFunctionType.Copy)
                            nc.sync.dma_start(
                                aggN[L][bg * GB * 128:(bg + 1) * GB * 128, :]
                                .rearrange("(q p) f -> p q f", p=128),
                                stg[:])

                nc.gpsimd.collective_compute(
                    "ReduceScatter", mybir.AluOpType.add,
                    replica_groups=[list(range(NCORES))],
                    ins=[aggN[L][:]], outs=[aggS[L][:]])

                # finish: self-loop + dsq + W + bias + relu (local shard)
                with tc.tile_pool(name=f"fv{L}", bufs=4) as fv, \
                     tc.tile_pool(name=f"fp{L}", bufs=2, space="PSUM") as fp:
                    nc.sync.dma_start(
                        agg_sb[L][:],
                        aggS[L].rearrange("(lb p) f -> p lb f", p=128))
                    for lb in range(NBC):
                        ta = fv.tile([128, D], f32, tag="ta")
                        nc.vector.tensor_add(ta[:],
                                             agg_sb[L][:, lb * D:(lb + 1) * D],
                                             hs_cur[:, lb * D:(lb + 1) * D])
                        tsc = fv.tile([128, D], bf16, tag="tsc")
                        nc.scalar.activation(tsc[:], ta[:],
                                             mybir.ActivationFunctionType.Copy,
                                             scale=ct["dsqk"][:, lb:lb + 1])
                        for pr in range(2):
                            tp = fp.tile([128, 128], bf16, tag="tp", space="PSUM")
                            nc.tensor.transpose(
                                tp[:], tsc[:, 128 * pr:128 * (pr + 1)], ident[:])
                            tps = fv.tile([128, 128], bf16, tag="tps")
                            nc.vector.tensor_copy(tps[:], tp[:])
                            wp = fp.tile([128, 128], f32, tag="wp", space="PSUM")
                            nc.tensor.matmul(wp[0:64, :], lhsT=gwd[0:64, :],
                                             rhs=tps[0:64, :], start=True, stop=True)
                            nc.tensor.matmul(wp[64:128, :], lhsT=gwd[64:128, :],
                                             rhs=tps[64:128, :], start=True, stop=True)
                            if L == 0:
                                hn = fv.tile([128, 128], bf16, tag="hn")
                                nc.scalar.activation(
                                    hn[:], wp[:],
                                    mybir.ActivationFunctionType.Relu,
                                    bias=gbs[:, 0:1])
                                tb = fp.tile([128, 128], bf16, tag="tb",
                                             space="PSUM")
                                nc.tensor.transpose(tb[:], hn[:], ident[:])
                                nc.vector.tensor_scalar_mul(
                                    hs1[:, lb * D + 128 * pr:
                                        lb * D + 128 * (pr + 1)],
                                    tb[:], ct["dsqk"][:, lb:lb + 1])
                            else:
                                h4 = fv.tile([128, 128], bf16, tag="h4")
                                nc.scalar.activation(
                                    h4[:], wp[:],
                                    mybir.ActivationFunctionType.Relu,
                                    bias=gbs[:, 0:1])
                                yp = fp.tile([128, 2], f32, tag="yp",
                                             space="PSUM")
                                nc.tensor.matmul(yp[:], lhsT=h4[:],
                                                 rhs=ct["wod2"][:],
                                                 start=True, stop=True)
                                nc.vector.tensor_scalar_add(
                                    y_nb[:, lb * 4 + pr * 2: lb * 4 + pr * 2 + 2],
                                    yp[:], bo_f)
                    if L == 0:
                        nc.sync.dma_start(
                            table1.rearrange("(lb p) f -> p lb f", p=128),
                            hs1[:])
            nc.sync.dma_start(y_ext[:], y_nb[:])
    nc.compile()
    return nc


def _run(inputs):
    from concourse.bass_utils import run_bass_kernel_spmd

    in_maps, slots, NGI, bo_f = _host_prep(
        inputs["x"], inputs["edge_index"], inputs["w1"], inputs["b1"],
        inputs["w2"], inputs["b2"], inputs["gw1"], inputs["gb1"],
        inputs["gw2"], inputs["gb2"], inputs["wo"], inputs["bo"])

    key = (hash(tuple(slots)), NGI)
    if key not in _cache:
        _cache[key] = _build(slots, NGI, bo_f)
    nc = _cache[key]

    res = run_bass_kernel_spmd(nc, in_maps, list(range(8)))
    y = np.zeros((B, N), dtype=np.float32)
    for k in range(NCORES):
        y_nb = res.results[k]["y"]          # [128, 4*NBC]
        for lb in range(NBC):
            lo = k * NSH + lb * 128
            hi = min(lo + 128, N)
            if hi <= lo:
                continue
            for s in range(B):
                y[s, lo:hi] = y_nb[: hi - lo, lb * 4 + s]
    return y


def kernel(**inputs):
    return _run(inputs)


# revision 26
# speedup vs baseline: 1.3857x; 1.0171x over previous
"""GraphWaveNet kernel for Trainium2 (Bass/Tile), 8 NeuronCores.

v2: edge sharding by SOURCE block + ReduceScatter (no AllGathers).

Algorithm notes:
- Only t=11 survives the final 1x1 conv; GCN doesn't mix time. So conv
  stack evaluated at t in {10,11} only, GCN on B=4 slices (D=256 cols =
  4 slices x 64 feats).
- GCN: with Hs = dsq*h, agg_n = dsq_n * (sum_{e->n} Hs[src_e] + Hs[n]).
- Sharding: core k owns node rows [1280k, 1280(k+1)). Conv computes the
  local shard of the Hs table (no collective). Edges are assigned to the
  core owning their SRC: gathers hit only the local table shard.
  Scatter (one-hot P matmuls into per-dst-block PSUM) produces a partial
  aggregate over ALL 10240 nodes; one f32 ReduceScatter per layer sums
  partials and hands each core its own 1280-row slice. Self-loop + W +
  bias + relu are then local.
- SPMD uniformity: slot schedule per dst block = max over cores of
  ceil(edges/128) (program identical across cores; per-core eidx/P data
  differ, padded with index-0 gathers and zero P columns).
- Gathers: 16 slots (2048 edges) per indirect DMA to amortize SWDGE
  descriptor-gen overhead (~1us/instruction on the Pool engine).
"""

import sys

sys.path.insert(0, "/opt/trn_rl_repo")

import numpy as np
import ml_dtypes

B, T, N, FIN, H, E = 4, 12, 10000, 2, 64, 80000
NCORES = 8
NB80 = 80                 # dst blocks of 128 nodes
NP = NB80 * 128           # padded node count (10240)
NSH = NP // NCORES        # node rows per core (1280)
NBC = NB80 // NCORES      # node blocks per core (10)
D = 4 * H                 # 256 = 4 slices x 64 feats
SPG = 8                   # slots (of 128 edges) per dma_gather (1024 idxs)
NIG = SPG * 128           # indices per gather instruction
ICPG = NIG // 16          # idx tile columns per gather instruction (64)

_cache = {}


def _host_prep(x, edge_index, w1, b1, w2, b2, gw1, gb1, gw2, gb2, wo, bo):
    x = np.asarray(x, np.float32)
    src = np.asarray(edge_index[0]).astype(np.int64)
    dst = np.asarray(edge_index[1]).astype(np.int64)

    deg = np.bincount(dst, minlength=N).astype(np.float64) + 1.0
    dsq = (deg ** -0.5).astype(np.float32)
    dsq_pad = np.ones(NP, dtype=np.float32)
    dsq_pad[:N] = dsq

    # ---- per-core edge partition by src owner, dst-sorted
    owner = src // NSH
    es_k, ed_k, cnt = [], [], np.zeros((NCORES, NB80), np.int64)
    for k in range(NCORES):
        m = owner == k
        es, ed = src[m], dst[m]
        o = np.argsort(ed, kind="stable")
        es, ed = es[o], ed[o]
        es_k.append(es)
        ed_k.append(ed)
        cnt[k] = np.bincount(ed // 128, minlength=NB80)

    S_b = np.maximum(1, (cnt + 127) // 128).max(axis=0)   # slots per block
    slots = []                                            # (block, first, last)
    for b in range(NB80):
        for j in range(int(S_b[b])):
            slots.append((b, j == 0, j == int(S_b[b]) - 1))
    TOT = len(slots)
    NGI = (TOT + SPG - 1) // SPG

    # eidx layout (dma_gather ucode contract, queue 0): within gather
    # instruction gi, flat index j in [0, NIG) lives at SBUF position
    # [16 + (j % 16), gi * ICPG + j // 16]; edge j lands at out[j%128, j//128].
    eidx_all = np.zeros((NCORES, 128, NGI * ICPG), np.int16)
    P_all = np.zeros((NCORES, 128, TOT * 128), np.float32)
    for k in range(NCORES):
        es, ed = es_k[k], ed_k[k]
        bounds = np.searchsorted(ed, np.arange(NB80 + 1) * 128)
        si = 0
        for b in range(NB80):
            e0, e1 = int(bounds[b]), int(bounds[b + 1])
            for j in range(int(S_b[b])):
                a = e0 + j * 128
                bb = min(a + 128, e1)
                ne = max(0, bb - a)
                if ne:
                    gi, c = divmod(si, SPG)
                    jj = c * 128 + np.arange(ne)        # flat idx within instr
                    eidx_all[k, 16 + (jj % 16), gi * ICPG + jj // 16] = \
                        (es[a:bb] - k * NSH).astype(np.int16)
                    P_all[k, np.arange(ne), si * 128 + (ed[a:bb] - b * 128)] = 1.0
                si += 1
    P_all = P_all.astype(ml_dtypes.bfloat16)

    # ---- conv input: per block 8 rows (t,c) for t in {9,10,11} + 2 zero rows,
    # cols = 4 slices x 128 nodes
    xpad = np.zeros((B, 3, FIN, NP), np.float32)
    xpad[:, :, :, :N] = x[:, 9:12, :, :].transpose(0, 1, 3, 2)  # [s, ti, c, n]
    xv = xpad.reshape(B, 6, NCORES, NBC, 128)                   # [s, row, k, blk, p]
    xt_all = np.zeros((NCORES, 8, NBC * 4 * 128), np.float32)
    xt_all[:, :6] = xv.transpose(2, 1, 3, 0, 4).reshape(NCORES, 6, NBC * 4 * 128)
    xt_all = xt_all.astype(ml_dtypes.bfloat16)

    dsqk_all = dsq_pad.reshape(NCORES, NBC, 128).transpose(0, 2, 1).copy()

    # ---- weights
    W1m = np.zeros((6, 64), np.float32)
    for kk in range(3):
        for c in range(FIN):
            W1m[2 * kk + c, :] = w1[:, c, 0, kk]
    W1ab = np.zeros((8, 128), np.float32)
    W1ab[0:6, 0:64] = W1m          # A: t10 (taps t9,t10,t11)
    W1ab[2:8, 64:128] = W1m        # B: t11 (taps t10,t11,t12=pad)
    W1ab = W1ab.astype(ml_dtypes.bfloat16)

    W2m = np.zeros((128, 64), np.float32)
    W2m[:64, :] = w2[:, :, 0, 0].T
    W2m[64:, :] = w2[:, :, 0, 1].T
    W2m = W2m.astype(ml_dtypes.bfloat16)

    b1s = np.concatenate([b1, b1]).reshape(128, 1).astype(np.float32)
    b2c = np.asarray(b2, np.float32).reshape(64, 1)
    gb1s = np.concatenate([gb1, gb1]).reshape(128, 1).astype(np.float32)
    gb2s = np.concatenate([gb2, gb2]).reshape(128, 1).astype(np.float32)
    gwd1 = np.concatenate([gw1, gw1], axis=0).astype(ml_dtypes.bfloat16)
    gwd2 = np.concatenate([gw2, gw2], axis=0).astype(ml_dtypes.bfloat16)
    wov = np.asarray(wo, np.float32)[0, :, 0, 0]
    wod2 = np.zeros((128, 2), np.float32)
    wod2[0:64, 0] = wov
    wod2[64:128, 1] = wov
    wod2 = wod2.astype(ml_dtypes.bfloat16)

    shared = {
        "W1ab": W1ab, "W2m": W2m, "b1s": b1s, "b2c": b2c,
        "gb1s": gb1s, "gb2s": gb2s, "gwd1": gwd1, "gwd2": gwd2, "wod2": wod2,
    }
    in_maps = []
    for k in range(NCORES):
        m = dict(shared)
        m["xt"] = xt_all[k]
        m["eidx"] = eidx_all[k]
        m["P"] = P_all[k]
        m["dsqk"] = dsqk_all[k]
        in_maps.append(m)
    return in_maps, slots, NGI, float(np.asarray(bo).reshape(-1)[0])


def _build(slots, NGI, bo_f):
    from concourse import bass, bacc, tile
    from concourse.masks import make_identity
    import mybir

    f32, bf16, i16 = mybir.dt.float32, mybir.dt.bfloat16, mybir.dt.int16
    f32r = mybir.dt.float32r
    TOT = len(slots)
    COLS = NGI * ICPG

    nc = bacc.Bacc("TRN2", target_bir_lowering=False, debug=False, num_devices=8,
                   dynamic_dma_scratch_size=65536)

    ext = {}
    for name, shape, dt in [
        ("xt", [8, NBC * 512], bf16), ("W1ab", [8, 128], bf16),
        ("W2m", [128, 64], bf16), ("b1s", [128, 1], f32), ("b2c", [64, 1], f32),
        ("gb1s", [128, 1], f32), ("gb2s", [128, 1], f32),
        ("gwd1", [128, 64], bf16), ("gwd2", [128, 64], bf16),
        ("wod2", [128, 2], bf16), ("dsqk", [128, NBC], f32),
        ("eidx", [128, COLS], i16), ("P", [128, TOT * 128], bf16),
    ]:
        ext[name] = nc.dram_tensor(name, shape, dt, kind="ExternalInput").ap()
    y_ext = nc.dram_tensor("y", [128, 4 * NBC], f32, kind="ExternalOutput").ap()
    table0 = nc.dram_tensor("table0", [NSH, D], bf16).ap()
    table1 = nc.dram_tensor("table1", [NSH, D], bf16).ap()
    aggN = [nc.dram_tensor(f"aggN{L}", [NP, D], bf16).ap() for L in range(2)]
    aggS = [nc.dram_tensor(f"aggS{L}", [NSH, D], bf16).ap() for L in range(2)]

    with tile.TileContext(nc) as tc:
        with tc.tile_pool(name="const", bufs=1) as cp, \
             tc.tile_pool(name="hs", bufs=1) as hp:
            ct = {}
            for name in ("W1ab", "W2m", "b1s", "b2c", "dsqk", "gb1s",
                         "gb2s", "gwd1", "gwd2", "wod2", "eidx"):
                t = cp.tile(list(ext[name].shape), ext[name].dtype, tag=name)
                nc.sync.dma_start(t[:], ext[name][:])
                ct[name] = t
            # P (5MB) loaded after conv's input DMA is issued -- SP runs its
            # queue in order and P would otherwise delay the conv start; P is
            # first needed by the scatter matmuls ~45us in.
            Pt = cp.tile(list(ext["P"].shape), ext["P"].dtype, tag="P")
            ct["P"] = Pt
            ident = cp.tile([128, 128], bf16, tag="ident")
            make_identity(nc, ident[:])
            y_nb = cp.tile([128, 4 * NBC], f32, tag="ynb")

            hs0 = hp.tile([128, NBC * D], bf16, tag="hs0")
            hs1 = hp.tile([128, NBC * D], bf16, tag="hs1")
            agg_sb0 = hp.tile([128, NBC * D], bf16, tag="asb0")
            agg_sb1 = hp.tile([128, NBC * D], bf16, tag="asb1")
            agg_sb = [agg_sb0, agg_sb1]

            # ---- conv stage: local table0 shard = dsq * relu(conv2(relu(conv1 x)))
            with tc.tile_pool(name="cv", bufs=3) as vp, \
                 tc.tile_pool(name="cvp", bufs=2, space="PSUM") as pp:
                xts = vp.tile([8, NBC * 512], bf16, tag="xts")
                nc.sync.dma_start(xts[:], ext["xt"][:])
                nc.sync.dma_start(Pt[:], ext["P"][:])
                for blk in range(NBC):
                    ph1 = pp.tile([128, 512], f32, tag="ph1", space="PSUM")
                    nc.tensor.matmul(ph1[:], lhsT=ct["W1ab"][:],
                                     rhs=xts[:, blk * 512:(blk + 1) * 512],
                                     start=True, stop=True)
                    h1 = vp.tile([128, 512], bf16, tag="h1")
                    nc.vector.tensor_scalar(h1[:], ph1[:],
                                            ct["b1s"][:, 0:1], 0.0,
                                            mybir.AluOpType.add,
                                            mybir.AluOpType.max)
                    ph2 = pp.tile([64, 512], f32, tag="ph2", space="PSUM")
                    nc.tensor.matmul(ph2[:], lhsT=ct["W2m"][:], rhs=h1[:],
                                     start=True, stop=True)
                    h2 = vp.tile([64, 512], bf16, tag="h2")
                    nc.scalar.activation(h2[:], ph2[:],
                                         mybir.ActivationFunctionType.Relu,
                                         bias=ct["b2c"][:, 0:1])
                    for s in range(4):
                        ptp = pp.tile([128, 64], bf16, tag="ptp", space="PSUM")
                        nc.tensor.transpose(ptp[:], h2[:, s * 128:(s + 1) * 128],
                                            ident[0:64, 0:64])
                        nc.vector.tensor_scalar_mul(
                            hs0[:, blk * D + 64 * s: blk * D + 64 * (s + 1)],
                            ptp[:], ct["dsqk"][:, blk:blk + 1])
                nc.sync.dma_start(
                    table0.rearrange("(lb p) f -> p lb f", p=128), hs0[:])

            # ---- GCN layers
            for L in range(2):
                tbl = table0 if L == 0 else table1
                hs_cur = hs0 if L == 0 else hs1
                gwd = ct["gwd1"] if L == 0 else ct["gwd2"]
                gbs = ct["gb1s"] if L == 0 else ct["gb2s"]

                # scatter: partial aggregate over all NP dst rows
                # blocks grouped by 4: one PSUM tile [128, 4, 512] f32 --
                # each block's accumulator bank-aligned (matmul PSUM outputs
                # must start at a bank boundary); one copy + one DMA per group
                GB = 4
                with tc.tile_pool(name=f"g{L}", bufs=5) as gp, \
                     tc.tile_pool(name=f"st{L}", bufs=4) as sp, \
                     tc.tile_pool(name=f"sc{L}", bufs=3, space="PSUM") as qp:
                    g = None
                    pb4 = None
                    for i, (b, first, last) in enumerate(slots):
                        gi, j = divmod(i, SPG)
                        if j == 0:
                            g = gp.tile([128, SPG, D], bf16, tag="g")
                            nc.gpsimd.dma_gather(
                                g[:], tbl[:],
                                ct["eidx"][:, gi * ICPG:(gi + 1) * ICPG],
                                NIG, NIG, D)
                        bg, bo_ = divmod(b, GB)
                        if first and bo_ == 0:
                            pb4 = qp.tile([128, GB, D], f32, tag="pb4",
                                          space="PSUM")
                        nc.tensor.matmul(pb4[:, bo_, 0:D],
                                         lhsT=ct["P"][:, i * 128:(i + 1) * 128],
                                         rhs=g[:, j, :],
                                         start=first, stop=last)
                        if last and bo_ == GB - 1:
                            stg = sp.tile([128, GB * D], bf16, tag="stg")
                            if bg % 2 == 0:
                                nc.vector.tensor_copy(stg[:], pb4[:, :, 0:D])
                            else:
                                nc.scalar.activation(
                                    stg[:], pb4[:, :, 0:D],
                                    mybir.ActivationFunctionType.Copy)
                            nc.sync.dma_start(
                                aggN[L][bg * GB * 128:(bg + 1) * GB * 128, :]
                                .rearrange("(q p) f -> p q f", p=128),
                                stg[:])

                nc.gpsimd.collective_compute(
                    "ReduceScatter", mybir.AluOpType.add,
                    replica_groups=[list(range(NCORES))],
                    ins=[aggN[L][:]], outs=[aggS[L][:]])

                # finish: self-loop + dsq + W + bias + relu (local shard),
                # processed in block-pairs to halve instruction count
                with tc.tile_pool(name=f"fv{L}", bufs=4) as fv, \
                     tc.tile_pool(name=f"fp{L}", bufs=2, space="PSUM") as fp:
                    nc.sync.dma_start(
                        agg_sb[L][:],
                        aggS[L].rearrange("(lb p) f -> p lb f", p=128))
                    for pi in range(NBC // 2):
                        b0, b1 = 2 * pi, 2 * pi + 1
                        tsc2 = fv.tile([128, 2 * D], bf16, tag="tsc2")
                        for w, b in enumerate((b0, b1)):
                            ta = fv.tile([128, D], f32, tag="ta")
                            nc.vector.tensor_add(ta[:],
                                                 agg_sb[L][:, b * D:(b + 1) * D],
                                                 hs_cur[:, b * D:(b + 1) * D])
                            if w == 0:
                                nc.scalar.activation(
                                    tsc2[:, w * D:(w + 1) * D], ta[:],
                                    mybir.ActivationFunctionType.Copy,
                                    scale=ct["dsqk"][:, b:b + 1])
                            else:
                                nc.vector.tensor_scalar_mul(
                                    tsc2[:, w * D:(w + 1) * D], ta[:],
                                    ct["dsqk"][:, b:b + 1])
                        tp4 = fp.tile([128, 512], bf16, tag="tp4", space="PSUM")
                        for c in range(4):   # chunk c = (block w=c//2, pr=c%2)
                            nc.tensor.transpose(
                                tp4[:, c * 128:(c + 1) * 128],
                                tsc2[:, c * 128:(c + 1) * 128], ident[:])
                        tps4 = fv.tile([128, 512], bf16, tag="tps4")
                        nc.vector.tensor_copy(tps4[:], tp4[:])
                        wp4 = fp.tile([128, 512], f32, tag="wp4", space="PSUM")
                        nc.tensor.matmul(wp4[0:64, :], lhsT=gwd[0:64, :],
                                         rhs=tps4[0:64, :], start=True, stop=True)
                        nc.tensor.matmul(wp4[64:128, :], lhsT=gwd[64:128, :],
                                         rhs=tps4[64:128, :], start=True, stop=True)
                        h44 = fv.tile([128, 512], bf16, tag="h44")
                        nc.scalar.activation(h44[:], wp4[:],
                                             mybir.ActivationFunctionType.Relu,
                                             bias=gbs[:, 0:1])
                        if L == 0:
                            tb4 = fp.tile([128, 512], bf16, tag="tb4",
                                          space="PSUM")
                            for c in range(4):
                                nc.tensor.transpose(
                                    tb4[:, c * 128:(c + 1) * 128],
                                    h44[:, c * 128:(c + 1) * 128], ident[:])
                            for c in range(4):
                                w, pr = divmod(c, 2)
                                b = 2 * pi + w
                                if c % 2 == 0:
                                    nc.vector.tensor_scalar_mul(
                                        hs1[:, b * D + 128 * pr:
                                            b * D + 128 * (pr + 1)],
                                        tb4[:, c * 128:(c + 1) * 128],
                                        ct["dsqk"][:, b:b + 1])
                                else:
                                    nc.scalar.activation(
                                        hs1[:, b * D + 128 * pr:
                                            b * D + 128 * (pr + 1)],
                                        tb4[:, c * 128:(c + 1) * 128],
                                        mybir.ActivationFunctionType.Copy,
                                        scale=ct["dsqk"][:, b:b + 1])
                        else:
                            yp4 = fp.tile([128, 8], f32, tag="yp4",
                                          space="PSUM")
                            for c in range(4):
                                nc.tensor.matmul(
                                    yp4[:, c * 2:(c + 1) * 2],
                                    lhsT=h44[:, c * 128:(c + 1) * 128],
                                    rhs=ct["wod2"][:],
                                    start=True, stop=True)
                            nc.vector.tensor_scalar_add(
                                y_nb[:, 8 * pi:8 * pi + 8], yp4[:], bo_f)
                    if L == 0:
                        nc.sync.dma_start(
                            table1.rearrange("(lb p) f -> p lb f", p=128),
                            hs1[:])
            nc.sync.dma_start(y_ext[:], y_nb[:])
    nc.compile()
    return nc


def _run(inputs):
    from concourse.bass_utils import run_bass_kernel_spmd

    in_maps, slots, NGI, bo_f = _host_prep(
        inputs["x"], inputs["edge_index"], inputs["w1"], inputs["b1"],
        inputs["w2"], inputs["b2"], inputs["gw1"], inputs["gb1"],
        inputs["gw2"], inputs["gb2"], inputs["wo"], inputs["bo"])

    key = (hash(tuple(slots)), NGI)
    if key not in _cache:
        _cache[key] = _build(slots, NGI, bo_f)
    nc = _cache[key]

    res = run_bass_kernel_spmd(nc, in_maps, list(range(8)))
    y = np.zeros((B, N), dtype=np.float32)
    for k in range(NCORES):
        y_nb = res.results[k]["y"]          # [128, 4*NBC]
        for lb in range(NBC):
            lo = k * NSH + lb * 128
            hi = min(lo + 128, N)
            if hi <= lo:
                continue
            for s in range(B):
                y[s, lo:hi] = y_nb[: hi - lo, lb * 4 + s]
    return y


def kernel(**inputs):
    return _run(inputs)
